# revision 2
# baseline (speedup 1.0000x reference)
"""Distributed causal multi-head attention (RoPE) for 8 TRN2 NeuronCores.

Problem: B=4, S=2048, D=2048, H=16 heads, DH=128.
Sharding: 2D — data-parallel over the 4 batches x tensor-parallel over 2
head-groups of 8 heads (Megatron-style: Wqkv column-sharded per head
group, Wo row-sharded).  Core c handles batch c//2, head group c%2.

The end-to-end time of kernel() is dominated by host<->device transfer
over the tunneled PJRT link (~30-80 MB/s), not by the on-device kernel
(~0.7 ms).  So the I/O contract is built to move each distinct byte
exactly once:
  - x:    core 2b+g ships HALF of batch b's xT (2 of 4 token-chunks);
          an on-device pair AllGather rebuilds the full [4,P,16,512].
  - Wqkv: each of the 4 cores of head-group g ships 1/4 of the group's
          pre-transposed Q/K blocks and V blocks; group AllGather
          ([0,2,4,6] / [1,3,5,7]) rebuilds the full group weights.
  - Wo:   same 4-way split by output-feature chunk, group AllGather.
  - RoPE tables: 8-way column split, all-core AllGather.
  - out:  each core's partial [S,D] goes through an on-device pair
          ReduceScatter (the Megatron all-reduce, fused with the
          batch-halving) so each core returns only its half of the
          summed batch output; the host just stitches slices.

All operands are bf16 (PE runs bf16 at 1 cycle/row with half the
SBUF/DMA footprint), accumulation in fp32 PSUM.  fp8 was measured and
rejected: attention output is a weighted mean, so per-element
quantization error lands ~1:1 in the output.

Per-core compute pipeline (~620us; PE sustains ~2.26GHz, 94% of peak):
  stage 1: QKV projection, streamed in consumption order on the single
           fast SP hardware DMA queue.  RoPE is fused into the PSUM
           eviction: ACT evicts psum to bf16 twice (straight +
           partition-rotated halves, the rotate_half swap), then three
           2x-mode DVE ops write the resident per-head q/k tiles.  V
           lands in vres with a ones column so PV yields the softmax
           denominator free.
  stage 2: query-chunk-outer, head-inner.  Per (qr, h): scoresT =
           K-tile.T x Q (causally trimmed), exp via ACT into bf16,
           triangular mask on the diagonal blocks only, PV via bf16
           matmuls with the fused ones column, reciprocal scale + PE
           transpose into resident zT tiles.  QK of job j+1 is emitted
           before PV of job j to hide the ACT exp latency.
  stage 3: out = sum_h zT_h.T x WoT_h, interleaved under stage 2.
"""

import sys

if '/opt/trn_rl_repo' not in sys.path:
    sys.path.insert(0, '/opt/trn_rl_repo')

import math
from concurrent.futures import ThreadPoolExecutor

import ml_dtypes
import numpy as np

B, S, D, H, DH = 4, 2048, 2048, 16, 128
BASE = 10000.0
P = 128
NT = S // P          # 16 token tiles of 128
NC512 = S // 512     # 4 token chunks of 512
NDM = D // P         # 16 d_model chunks
HG = 8               # heads per group
VW = P + 1           # v block width per head (128 + ones column)
SCALE = 1.0 / math.sqrt(DH)

PAIRS = [[0, 1], [2, 3], [4, 5], [6, 7]]       # (batch) pairs: same b, two head groups
GROUPS = [[0, 2, 4, 6], [1, 3, 5, 7]]          # head-group g = cores with c%2==g
ALL8 = [[0, 1, 2, 3, 4, 5, 6, 7]]

_CACHE = {}


def _build_program():
    import concourse.bacc as bacc
    import concourse.mybir as mybir
    from concourse.tile import TileContext
    from concourse.masks import make_identity

    F32 = mybir.dt.float32
    BF16 = mybir.dt.bfloat16
    EXP = mybir.ActivationFunctionType.Exp
    BYP = mybir.AluOpType.bypass
    ADD = mybir.AluOpType.add

    nc = bacc.Bacc('TRN2', target_bir_lowering=False, debug=False, num_devices=8)

    # ---- DRAM I/O (each core ships only its distinct shard) ----
    xT_in = nc.dram_tensor('xT_in', [2, P, NDM, 512], BF16, kind='ExternalInput').ap()
    wqk_in = nc.dram_tensor('wqk_in', [4, P, NDM, P], BF16, kind='ExternalInput').ap()
    wv_in = nc.dram_tensor('wv_in', [1, P, NDM, 256], BF16, kind='ExternalInput').ap()
    wo_in = nc.dram_tensor('wo_in', [1, P, HG, 512], BF16, kind='ExternalInput').ap()
    cs_in = nc.dram_tensor('cs_in', [2, P, 256], BF16, kind='ExternalInput').ap()
    maskT = nc.dram_tensor('maskT', [P, P], BF16, kind='ExternalInput').ap()
    out = nc.dram_tensor('out', [NT // 2, P, D], BF16, kind='ExternalOutput').ap()

    with TileContext(nc) as tc:
        with tc.tile_pool(name='dram', bufs=1, space='DRAM') as dram:
            # bounce copies (collectives can't touch I/O tensors) + gathered full tensors
            xb = dram.tile([2, P, NDM, 512], BF16)
            xT = dram.tile([NC512, P, NDM, 512], BF16)
            wqkb = dram.tile([4, P, NDM, P], BF16)
            wqkT = dram.tile([2 * HG, P, NDM, P], BF16)
            wvb = dram.tile([1, P, NDM, 256], BF16)
            wvT = dram.tile([4, P, NDM, 256], BF16)
            wob = dram.tile([1, P, HG, 512], BF16)
            woT = dram.tile([NC512, P, HG, 512], BF16)
            csb = dram.tile([2, P, 256], BF16)
            csg = dram.tile([16, P, 256], BF16)
            partial = dram.tile([NT, P, D], BF16)
            rso = dram.tile([NT // 2, P, D], BF16)

            nc.sync.dma_start(xb[:], xT_in[:])
            nc.sync.dma_start(wqkb[:], wqk_in[:])
            nc.sync.dma_start(wvb[:], wv_in[:])
            nc.sync.dma_start(wob[:], wo_in[:])
            nc.sync.dma_start(csb[:], cs_in[:])
            nc.gpsimd.collective_compute('AllGather', BYP, replica_groups=PAIRS,
                                         ins=[xb.opt()], outs=[xT.opt()])
            nc.gpsimd.collective_compute('AllGather', BYP, replica_groups=GROUPS,
                                         ins=[wqkb.opt()], outs=[wqkT.opt()])
            nc.gpsimd.collective_compute('AllGather', BYP, replica_groups=GROUPS,
                                         ins=[wvb.opt()], outs=[wvT.opt()])
            nc.gpsimd.collective_compute('AllGather', BYP, replica_groups=GROUPS,
                                         ins=[wob.opt()], outs=[woT.opt()])
            nc.gpsimd.collective_compute('AllGather', BYP, replica_groups=ALL8,
                                         ins=[csb.opt()], outs=[csg.opt()])

            with tc.tile_pool(name='res', bufs=1) as rpool:
                msk = rpool.tile([P, P], BF16)
                identb = rpool.tile([P, P], BF16)
                qres = [rpool.tile([P, S], BF16, name=f'q{h}') for h in range(HG)]
                kres = [rpool.tile([P, S], BF16, name=f'k{h}') for h in range(HG)]
                vres = rpool.tile([P, NT, HG * VW], BF16)

                # ================= stage 1: QKV projection =================
                with tc.tile_pool(name='s1x', bufs=1) as xpool, \
                     tc.tile_pool(name='s1w', bufs=2) as wpool, \
                     tc.tile_pool(name='s1e', bufs=3) as epool, \
                     tc.tile_pool(name='s1p', bufs=4, space='PSUM') as qpp, \
                     tc.tile_pool(name='s1pv', bufs=4, space='PSUM') as vpp:
                    xsb = xpool.tile([P, NC512, NDM, 512], BF16)
                    cos_sb = xpool.tile([P, S], BF16)
                    sin_sb = xpool.tile([P, S], BF16)
                    # Consumption-ordered input feed on the single fast
                    # hardware DMA queue (SP/sync).
                    nc.sync.dma_start(xsb[:, 0], xT[0])
                    w0 = wpool.tile([P, NDM, P], BF16, tag='w', name='w0')
                    nc.sync.dma_start(w0[:], wqkT[0])
                    nc.sync.dma_start(xsb[:, 1], xT[1])
                    for i in range(8):
                        nc.sync.dma_start(cos_sb[:, i * 256:(i + 1) * 256], csg[2 * i])
                        nc.sync.dma_start(sin_sb[:, i * 256:(i + 1) * 256], csg[2 * i + 1])
                    for tcn in range(2, NC512):
                        nc.sync.dma_start(xsb[:, tcn], xT[tcn])
                    nc.sync.dma_start(msk[:], maskT[:])
                    make_identity(nc, identb[:])
                    for h in range(HG):
                        nc.gpsimd.memset(vres[:, :, h * VW + P:h * VW + P + 1], 1.0)

                    for fb in range(2 * HG):
                        if fb == 0:
                            w = w0
                        else:
                            w = wpool.tile([P, NDM, P], BF16, tag='w', name=f'w{fb}')
                            nc.sync.dma_start(w[:], wqkT[fb])
                        dest = qres[fb] if fb < HG else kres[fb - HG]
                        for tcn in range(NC512):
                            ts = slice(tcn * 512, tcn * 512 + 512)
                            ps = qpp.tile([P, 512], F32, tag='pqk', name=f'pqk_{fb}_{tcn}')
                            for o in range(NDM):
                                nc.tensor.matmul(ps[:], w[:, o, :], xsb[:, tcn, o, :],
                                                 start=(o == 0), stop=(o == NDM - 1))
                            # RoPE fused eviction (sign folded into sinP).
                            psb = epool.tile([P, 512], BF16, tag='psb', name=f'psb_{fb}_{tcn}')
                            nc.scalar.copy(psb[:], ps[:])
                            psr = epool.tile([P, 512], BF16, tag='psr', name=f'psr_{fb}_{tcn}')
                            nc.scalar.copy(psr[0:64, :], ps[64:128, :])
                            nc.scalar.copy(psr[64:128, :], ps[0:64, :])
                            t1 = epool.tile([P, 512], BF16, tag='t1', name=f't1_{fb}_{tcn}')
                            t2 = epool.tile([P, 512], BF16, tag='t2', name=f't2_{fb}_{tcn}')
                            nc.vector.tensor_mul(t1[:], psb[:], cos_sb[:, ts])
                            nc.vector.tensor_mul(t2[:], psr[:], sin_sb[:, ts])
                            nc.vector.tensor_add(dest[:, ts], t1[:], t2[:])

                    # --- V blocks, token-major ---
                    for vc in range(4):
                        wv = wpool.tile([P, NDM, 256], BF16, tag='wv', name=f'wv{vc}')
                        nc.sync.dma_start(wv[:], wvT[vc])
                        for tt in range(NT):
                            psv = vpp.tile([P, 256], F32, tag='pv', name=f'pv_{vc}_{tt}')
                            for o in range(NDM):
                                nc.tensor.matmul(psv[:],
                                                 xsb[:, tt // 4, o,
                                                     (tt % 4) * P:(tt % 4 + 1) * P],
                                                 wv[:, o, :],
                                                 start=(o == 0), stop=(o == NDM - 1))
                            for j in range(2):
                                hv = 2 * vc + j
                                nc.scalar.copy(vres[:, tt, hv * VW:hv * VW + P],
                                               psv[:, j * P:(j + 1) * P])

                # ============ stage 2 + interleaved stage 3 ============
                with tc.tile_pool(name='res2', bufs=1) as r2pool, \
                     tc.tile_pool(name='s2st', bufs=2) as stpool, \
                     tc.tile_pool(name='s2z', bufs=4) as zpool, \
                     tc.tile_pool(name='s2os', bufs=4) as ospool, \
                     tc.tile_pool(name='s2p', bufs=3, space='PSUM') as spp, \
                     tc.tile_pool(name='s2pz', bufs=2, space='PSUM') as zpp, \
                     tc.tile_pool(name='s2pt', bufs=1, space='PSUM') as tpp, \
                     tc.tile_pool(name='s3p', bufs=2, space='PSUM') as opp:

                    zres = [r2pool.tile([P, S], BF16, name=f'z{h}') for h in range(HG)]
                    wo_sb = r2pool.tile([P, NC512, HG, 512], BF16)
                    for ec in range(NC512):
                        nc.sync.dma_start(wo_sb[:, ec], woT[ec])
                    st = {}
                    pend_t = []

                    def flush_t():
                        ph, pqa, pzsb = pend_t.pop(0)
                        ztp = tpp.tile([P, P], BF16, tag='ztp', name=f'ztp_{ph}_{pqa}')
                        nc.tensor.transpose(ztp[:], pzsb[:], identb[:])
                        if pqa % 2 == 0:
                            nc.vector.tensor_copy(zres[ph][:, pqa * P:(pqa + 1) * P], ztp[:])
                        else:
                            nc.scalar.copy(zres[ph][:, pqa * P:(pqa + 1) * P], ztp[:])

                    def emit_qk(qr, h):
                        base = qr * 512
                        tiles = []
                        for kt in range(4 * qr + 4):
                            d = kt - 4 * qr
                            off = 0 if d < 0 else P * d   # causal trim
                            sps = spp.tile([P, 512], F32, tag='sps',
                                           name=f'sps_{qr}_{h}_{kt}')
                            nc.tensor.matmul(sps[:, off:512],
                                             kres[h][:, kt * P:(kt + 1) * P],
                                             qres[h][:, base + off:base + 512],
                                             start=True, stop=True)
                            stt = stpool.tile([P, 512], BF16, tag=f'st{kt}',
                                              name=f'st_{qr}_{h}_{kt}')
                            nc.scalar.activation(stt[:, off:512], sps[:, off:512],
                                                 EXP, scale=SCALE)
                            if d >= 0:
                                # triangular mask on the diagonal 128-block only
                                nc.vector.tensor_mul(stt[:, off:off + P],
                                                     stt[:, off:off + P], msk[:])
                            tiles.append(stt)
                        st[(qr, h)] = tiles

                    def emit_pv(qr, h):
                        tiles = st.pop((qr, h))
                        for qs in range(4):
                            qa = 4 * qr + qs
                            zps = zpp.tile([P, VW], F32, tag='zps',
                                           name=f'zps_{qr}_{h}_{qs}')
                            for kt in range(qa + 1):
                                nc.tensor.matmul(zps[:],
                                                 tiles[kt][:, qs * P:(qs + 1) * P],
                                                 vres[:, kt, h * VW:(h + 1) * VW],
                                                 start=(kt == 0), stop=(kt == qa))
                            rcp = zpool.tile([P, 1], F32, tag='rcp',
                                             name=f'rcp_{qr}_{h}_{qs}')
                            nc.vector.reciprocal(rcp[:], zps[:, P:P + 1])
                            zsb = zpool.tile([P, P], BF16, tag='zsb',
                                             name=f'zsb_{qr}_{h}_{qs}')
                            nc.vector.tensor_scalar_mul(zsb[:], zps[:, 0:P], rcp[:])
                            # delay the transpose one step so the DVE epilogue
                            # hides under the next PV block's matmuls
                            pend_t.append((h, qa, zsb))
                            if len(pend_t) > 1:
                                flush_t()

                    def emit_s3(qr, ci):
                        ec, tl = divmod(ci, 4)
                        tt = 4 * qr + tl
                        es = slice(ec * 512, ec * 512 + 512)
                        pso = opp.tile([P, 512], F32, tag='pso', name=f'pso_{tt}_{ec}')
                        for h in range(HG):
                            nc.tensor.matmul(pso[:], zres[h][:, tt * P:(tt + 1) * P],
                                             wo_sb[:, ec, h, :],
                                             start=(h == 0), stop=(h == HG - 1))
                        osb = ospool.tile([P, 512], BF16, tag='osb',
                                          name=f'osb_{tt}_{ec}')
                        if (tt + ec) % 2 == 0:
                            nc.scalar.copy(osb[:], pso[:])
                        else:
                            nc.vector.tensor_copy(osb[:], pso[:])
                        nc.sync.dma_start(partial[tt][:, es], osb[:])

                    jobs = [(qr, h) for qr in range(NC512) for h in range(HG)]
                    emit_qk(*jobs[0])
                    for i, (qr, h) in enumerate(jobs):
                        if i + 1 < len(jobs):
                            emit_qk(*jobs[i + 1])
                        emit_pv(qr, h)
                        if qr >= 1:
                            emit_s3(qr - 1, 2 * h)
                            emit_s3(qr - 1, 2 * h + 1)
                    while pend_t:
                        flush_t()
                    for ci in range(16):
                        emit_s3(NC512 - 1, ci)

            # ---- Megatron all-reduce fused with batch halving ----
            nc.gpsimd.collective_compute('ReduceScatter', ADD, replica_groups=PAIRS,
                                         ins=[partial.opt()], outs=[rso.opt()])
            nc.sync.dma_start(out[:], rso[:])

    nc.compile()
    return nc


def _host_inputs(x, Wqkv, Wo):
    """Build the 8 per-core input maps (each core gets a distinct shard)."""
    BF = ml_dtypes.bfloat16
    # RoPE tables (f32 math, bf16 storage; sign folded into sinP rows 0:64)
    inv_freq = (1.0 / (BASE ** (np.arange(0, DH, 2, dtype=np.float32) / DH))).astype(np.float32)
    t = np.arange(S, dtype=np.float32)
    freqs = np.einsum('i,j->ij', t, inv_freq).astype(np.float32)   # [S, 64]
    emb = np.concatenate([freqs, freqs], axis=-1)                   # [S, 128]
    cosT = np.ascontiguousarray(np.cos(emb).T).astype(BF)           # [128, S]
    sinT = np.ascontiguousarray(np.sin(emb).T)
    sinT[0:64] = -sinT[0:64]
    sinP = sinT.astype(BF)

    # triangular causal mask [128, 128]: keep iff k_rel <= q_rel
    maskT = (np.arange(P)[:, None] <= np.arange(P)[None, :]).astype(BF)

    def xT_batch(b):
        # chunk-major [tcn, 128, 16, 512] so each chunk is one contiguous DMA
        return np.ascontiguousarray(
            x[b].T.reshape(NDM, P, NC512, 512).transpose(2, 1, 0, 3)).astype(BF)

    def wqkT_group(g):
        heads = range(HG * g, HG * g + HG)
        blocks = [Wqkv[h * DH:(h + 1) * DH] for h in heads] + \
                 [Wqkv[D + h * DH:D + (h + 1) * DH] for h in heads]
        return np.stack([
            np.ascontiguousarray(
                blk.T.reshape(NDM, P, P).transpose(1, 0, 2)).astype(BF)
            for blk in blocks
        ])                                                       # [16, 128, 16, 128]

    def wvT_group(g):
        heads = range(HG * g, HG * g + HG)
        Wv = np.concatenate([Wqkv[2 * D + h * DH:2 * D + (h + 1) * DH] for h in heads])
        return np.ascontiguousarray(
            Wv.T.reshape(NDM, P, 4, 256).transpose(2, 1, 0, 3)).astype(BF)

    def woT_group(g):
        Wog = Wo[:, g * HG * DH:(g + 1) * HG * DH]               # [D, 1024]
        return np.ascontiguousarray(
            Wog.T.reshape(HG, P, NC512, 512).transpose(2, 1, 0, 3)).astype(BF)

    with ThreadPoolExecutor(max_workers=8) as ex:
        xTs = list(ex.map(xT_batch, range(B)))
        wqkTs = list(ex.map(wqkT_group, range(2)))
        wvTs = list(ex.map(wvT_group, range(2)))
        woTs = list(ex.map(woT_group, range(2)))

    in_maps = []
    for c in range(8):
        b, g = c // 2, c % 2
        p = b                        # rank slot of this core within its group AG
        cs = np.stack([cosT[:, c * 256:(c + 1) * 256],
                       sinP[:, c * 256:(c + 1) * 256]])
        in_maps.append({
            'xT_in': xTs[b][2 * g:2 * g + 2],
            'wqk_in': wqkTs[g][4 * p:4 * p + 4],
            'wv_in': wvTs[g][p:p + 1],
            'wo_in': woTs[g][p:p + 1],
            'cs_in': cs,
            'maskT': maskT,
        })
    return in_maps


def kernel(x, Wqkv, Wo):
    from concourse.bass_utils import run_bass_kernel_spmd

    if 'nc' not in _CACHE:
        _CACHE['nc'] = _build_program()
    nc = _CACHE['nc']

    in_maps = _host_inputs(np.asarray(x, dtype=np.float32),
                           np.asarray(Wqkv, dtype=np.float32),
                           np.asarray(Wo, dtype=np.float32))
    res = run_bass_kernel_spmd(nc, in_maps, core_ids=list(range(8)))
    full = np.empty((B, S, D), dtype=np.float32)
    for b in range(B):
        full[b, 0:S // 2] = res.results[2 * b]['out'].reshape(S // 2, D).astype(np.float32)
        full[b, S // 2:S] = res.results[2 * b + 1]['out'].reshape(S // 2, D).astype(np.float32)
    return full


# revision 5
# speedup vs baseline: 1.0029x; 1.0029x over previous
"""Distributed causal multi-head attention (RoPE) for 8 TRN2 NeuronCores.

Problem: B=4, S=2048, D=2048, H=16 heads, DH=128.
Sharding: 2D — data-parallel over the 4 batches x tensor-parallel over 2
head-groups of 8 heads (Megatron-style: Wqkv column-sharded per head
group, Wo row-sharded).  Core c handles batch c//2, head group c%2.

The end-to-end time of kernel() is dominated by host<->device transfer
over the tunneled PJRT link (~30-80 MB/s), not by the on-device kernel
(~0.7 ms).  So the I/O contract is built to move each distinct byte
exactly once:
  - x:    core 2b+g ships HALF of batch b's xT (2 of 4 token-chunks);
          an on-device pair AllGather rebuilds the full [4,P,16,512].
  - Wqkv: each of the 4 cores of head-group g ships 1/4 of the group's
          pre-transposed Q/K blocks and V blocks; group AllGather
          ([0,2,4,6] / [1,3,5,7]) rebuilds the full group weights.
  - Wo:   same 4-way split by output-feature chunk, group AllGather.
  - RoPE tables: 8-way column split, all-core AllGather.
  - out:  each core's partial [S,D] goes through an on-device pair
          ReduceScatter (the Megatron all-reduce, fused with the
          batch-halving) so each core returns only its half of the
          summed batch output; the host just stitches slices.

All operands are bf16 (PE runs bf16 at 1 cycle/row with half the
SBUF/DMA footprint), accumulation in fp32 PSUM.  fp8 was measured and
rejected: attention output is a weighted mean, so per-element
quantization error lands ~1:1 in the output.

Per-core compute pipeline (~620us; PE sustains ~2.26GHz, 94% of peak):
  stage 1: QKV projection, streamed in consumption order on the single
           fast SP hardware DMA queue.  RoPE is fused into the PSUM
           eviction: ACT evicts psum to bf16 twice (straight +
           partition-rotated halves, the rotate_half swap), then three
           2x-mode DVE ops write the resident per-head q/k tiles.  V
           lands in vres with a ones column so PV yields the softmax
           denominator free.
  stage 2: query-chunk-outer, head-inner.  Per (qr, h): scoresT =
           K-tile.T x Q (causally trimmed), exp via ACT into bf16,
           triangular mask on the diagonal blocks only, PV via bf16
           matmuls with the fused ones column, reciprocal scale + PE
           transpose into resident zT tiles.  QK of job j+1 is emitted
           before PV of job j to hide the ACT exp latency.
  stage 3: out = sum_h zT_h.T x WoT_h, interleaved under stage 2.
"""

import sys

if '/opt/trn_rl_repo' not in sys.path:
    sys.path.insert(0, '/opt/trn_rl_repo')

import math
from concurrent.futures import ThreadPoolExecutor

import ml_dtypes
import numpy as np

B, S, D, H, DH = 4, 2048, 2048, 16, 128
BASE = 10000.0
P = 128
NT = S // P          # 16 token tiles of 128
NC512 = S // 512     # 4 token chunks of 512
NDM = D // P         # 16 d_model chunks
HG = 8               # heads per group
VW = P + 1           # v block width per head (128 + ones column)
SCALE = 1.0 / math.sqrt(DH)

PAIRS = [[0, 1], [2, 3], [4, 5], [6, 7]]       # (batch) pairs: same b, two head groups
GROUPS = [[0, 2, 4, 6], [1, 3, 5, 7]]          # head-group g = cores with c%2==g
ALL8 = [[0, 1, 2, 3, 4, 5, 6, 7]]

_CACHE = {}


def _build_program():
    import concourse.bacc as bacc
    import concourse.mybir as mybir
    from concourse.tile import TileContext
    from concourse.masks import make_identity

    F32 = mybir.dt.float32
    BF16 = mybir.dt.bfloat16
    EXP = mybir.ActivationFunctionType.Exp
    BYP = mybir.AluOpType.bypass
    ADD = mybir.AluOpType.add

    nc = bacc.Bacc('TRN2', target_bir_lowering=False, debug=False, num_devices=8)

    # ---- DRAM I/O (each core ships only its distinct shard) ----
    xT_in = nc.dram_tensor('xT_in', [2, P, NDM, 512], BF16, kind='ExternalInput').ap()
    wqk_in = nc.dram_tensor('wqk_in', [4, P, NDM, P], BF16, kind='ExternalInput').ap()
    wv_in = nc.dram_tensor('wv_in', [1, P, NDM, 256], BF16, kind='ExternalInput').ap()
    wo_in = nc.dram_tensor('wo_in', [1, P, HG, 512], BF16, kind='ExternalInput').ap()
    cs_in = nc.dram_tensor('cs_in', [2, P, 256], BF16, kind='ExternalInput').ap()
    maskT = nc.dram_tensor('maskT', [P, P], BF16, kind='ExternalInput').ap()
    out = nc.dram_tensor('out', [NT // 2, P, D], BF16, kind='ExternalOutput').ap()

    with TileContext(nc) as tc:
        with tc.tile_pool(name='dram', bufs=1, space='DRAM') as dram:
            # bounce copies (collectives can't touch I/O tensors) + gathered full tensors
            xb = dram.tile([2, P, NDM, 512], BF16)
            xT = dram.tile([NC512, P, NDM, 512], BF16)
            wqkb = dram.tile([4, P, NDM, P], BF16)
            wqkT = dram.tile([2 * HG, P, NDM, P], BF16)
            wvb = dram.tile([1, P, NDM, 256], BF16)
            wvT = dram.tile([4, P, NDM, 256], BF16)
            wob = dram.tile([1, P, HG, 512], BF16)
            woT = dram.tile([NC512, P, HG, 512], BF16)
            csb = dram.tile([2, P, 256], BF16)
            csg = dram.tile([16, P, 256], BF16)
            partial = dram.tile([NT, P, D], BF16)
            rso = dram.tile([NT // 2, P, D], BF16)

            nc.sync.dma_start(xb[:], xT_in[:])
            nc.sync.dma_start(wqkb[:], wqk_in[:])
            nc.sync.dma_start(wvb[:], wv_in[:])
            nc.sync.dma_start(wob[:], wo_in[:])
            nc.sync.dma_start(csb[:], cs_in[:])
            nc.gpsimd.collective_compute('AllGather', BYP, replica_groups=PAIRS,
                                         ins=[xb.opt()], outs=[xT.opt()])
            nc.gpsimd.collective_compute('AllGather', BYP, replica_groups=GROUPS,
                                         ins=[wqkb.opt()], outs=[wqkT.opt()])
            nc.gpsimd.collective_compute('AllGather', BYP, replica_groups=GROUPS,
                                         ins=[wvb.opt()], outs=[wvT.opt()])
            nc.gpsimd.collective_compute('AllGather', BYP, replica_groups=GROUPS,
                                         ins=[wob.opt()], outs=[woT.opt()])
            nc.gpsimd.collective_compute('AllGather', BYP, replica_groups=ALL8,
                                         ins=[csb.opt()], outs=[csg.opt()])

            with tc.tile_pool(name='res', bufs=1) as rpool:
                msk = rpool.tile([P, P], BF16)
                identb = rpool.tile([P, P], BF16)
                qres = [rpool.tile([P, S], BF16, name=f'q{h}') for h in range(HG)]
                kres = [rpool.tile([P, S], BF16, name=f'k{h}') for h in range(HG)]
                vres = rpool.tile([P, NT, HG * VW], BF16)

                # ================= stage 1: QKV projection =================
                with tc.tile_pool(name='s1x', bufs=1) as xpool, \
                     tc.tile_pool(name='s1w', bufs=2) as wpool, \
                     tc.tile_pool(name='s1e', bufs=3) as epool, \
                     tc.tile_pool(name='s1p', bufs=4, space='PSUM') as qpp, \
                     tc.tile_pool(name='s1pv', bufs=4, space='PSUM') as vpp:
                    xsb = xpool.tile([P, NC512, NDM, 512], BF16)
                    cos_sb = xpool.tile([P, S], BF16)
                    sin_sb = xpool.tile([P, S], BF16)
                    # Consumption-ordered input feed on the single fast
                    # hardware DMA queue (SP/sync).
                    nc.sync.dma_start(xsb[:, 0], xT[0])
                    w0 = wpool.tile([P, NDM, P], BF16, tag='w', name='w0')
                    nc.sync.dma_start(w0[:], wqkT[0])
                    nc.sync.dma_start(xsb[:, 1], xT[1])
                    for i in range(8):
                        nc.sync.dma_start(cos_sb[:, i * 256:(i + 1) * 256], csg[2 * i])
                        nc.sync.dma_start(sin_sb[:, i * 256:(i + 1) * 256], csg[2 * i + 1])
                    for tcn in range(2, NC512):
                        nc.sync.dma_start(xsb[:, tcn], xT[tcn])
                    nc.sync.dma_start(msk[:], maskT[:])
                    make_identity(nc, identb[:])
                    for h in range(HG):
                        nc.gpsimd.memset(vres[:, :, h * VW + P:h * VW + P + 1], 1.0)

                    for fb in range(2 * HG):
                        if fb == 0:
                            w = w0
                        else:
                            w = wpool.tile([P, NDM, P], BF16, tag='w', name=f'w{fb}')
                            nc.sync.dma_start(w[:], wqkT[fb])
                        dest = qres[fb] if fb < HG else kres[fb - HG]
                        for tcn in range(NC512):
                            ts = slice(tcn * 512, tcn * 512 + 512)
                            ps = qpp.tile([P, 512], F32, tag='pqk', name=f'pqk_{fb}_{tcn}')
                            for o in range(NDM):
                                nc.tensor.matmul(ps[:], w[:, o, :], xsb[:, tcn, o, :],
                                                 start=(o == 0), stop=(o == NDM - 1))
                            # RoPE fused eviction (sign folded into sinP).
                            psb = epool.tile([P, 512], BF16, tag='psb', name=f'psb_{fb}_{tcn}')
                            nc.scalar.copy(psb[:], ps[:])
                            psr = epool.tile([P, 512], BF16, tag='psr', name=f'psr_{fb}_{tcn}')
                            nc.scalar.copy(psr[0:64, :], ps[64:128, :])
                            nc.scalar.copy(psr[64:128, :], ps[0:64, :])
                            t1 = epool.tile([P, 512], BF16, tag='t1', name=f't1_{fb}_{tcn}')
                            t2 = epool.tile([P, 512], BF16, tag='t2', name=f't2_{fb}_{tcn}')
                            nc.vector.tensor_mul(t1[:], psb[:], cos_sb[:, ts])
                            nc.vector.tensor_mul(t2[:], psr[:], sin_sb[:, ts])
                            nc.vector.tensor_add(dest[:, ts], t1[:], t2[:])

                    # --- V blocks, token-major ---
                    for vc in range(4):
                        wv = wpool.tile([P, NDM, 256], BF16, tag='wv', name=f'wv{vc}')
                        nc.sync.dma_start(wv[:], wvT[vc])
                        for tt in range(NT):
                            psv = vpp.tile([P, 256], F32, tag='pv', name=f'pv_{vc}_{tt}')
                            for o in range(NDM):
                                nc.tensor.matmul(psv[:],
                                                 xsb[:, tt // 4, o,
                                                     (tt % 4) * P:(tt % 4 + 1) * P],
                                                 wv[:, o, :],
                                                 start=(o == 0), stop=(o == NDM - 1))
                            for j in range(2):
                                hv = 2 * vc + j
                                nc.scalar.copy(vres[:, tt, hv * VW:hv * VW + P],
                                               psv[:, j * P:(j + 1) * P])

                # ============ stage 2 + interleaved stage 3 ============
                with tc.tile_pool(name='res2', bufs=1) as r2pool, \
                     tc.tile_pool(name='s2st', bufs=2) as stpool, \
                     tc.tile_pool(name='s2z', bufs=4) as zpool, \
                     tc.tile_pool(name='s2os', bufs=4) as ospool, \
                     tc.tile_pool(name='s2p', bufs=3, space='PSUM') as spp, \
                     tc.tile_pool(name='s2pz', bufs=2, space='PSUM') as zpp, \
                     tc.tile_pool(name='s2pt', bufs=1, space='PSUM') as tpp, \
                     tc.tile_pool(name='s3p', bufs=2, space='PSUM') as opp:

                    zres = [r2pool.tile([P, S], BF16, name=f'z{h}') for h in range(HG)]
                    wo_sb = r2pool.tile([P, NC512, HG, 512], BF16)
                    for ec in range(NC512):
                        nc.sync.dma_start(wo_sb[:, ec], woT[ec])
                    st = {}
                    pend_t = []

                    def flush_t():
                        ph, pqa, pzsb = pend_t.pop(0)
                        ztp = tpp.tile([P, P], BF16, tag='ztp', name=f'ztp_{ph}_{pqa}')
                        nc.tensor.transpose(ztp[:], pzsb[:], identb[:])
                        if pqa % 2 == 0:
                            nc.vector.tensor_copy(zres[ph][:, pqa * P:(pqa + 1) * P], ztp[:])
                        else:
                            nc.scalar.copy(zres[ph][:, pqa * P:(pqa + 1) * P], ztp[:])

                    def emit_qk(qr, h):
                        base = qr * 512
                        tiles = []
                        for kt in range(4 * qr + 4):
                            d = kt - 4 * qr
                            off = 0 if d < 0 else P * d   # causal trim
                            sps = spp.tile([P, 512], F32, tag='sps',
                                           name=f'sps_{qr}_{h}_{kt}')
                            nc.tensor.matmul(sps[:, off:512],
                                             kres[h][:, kt * P:(kt + 1) * P],
                                             qres[h][:, base + off:base + 512],
                                             start=True, stop=True)
                            stt = stpool.tile([P, 512], BF16, tag=f'st{kt}',
                                              name=f'st_{qr}_{h}_{kt}')
                            nc.scalar.activation(stt[:, off:512], sps[:, off:512],
                                                 EXP, scale=SCALE)
                            if d >= 0:
                                # triangular mask on the diagonal 128-block only
                                nc.vector.tensor_mul(stt[:, off:off + P],
                                                     stt[:, off:off + P], msk[:])
                            tiles.append(stt)
                        st[(qr, h)] = tiles

                    def emit_pv(qr, h):
                        tiles = st.pop((qr, h))
                        for qs in range(4):
                            qa = 4 * qr + qs
                            zps = zpp.tile([P, VW], F32, tag='zps',
                                           name=f'zps_{qr}_{h}_{qs}')
                            for kt in range(qa + 1):
                                nc.tensor.matmul(zps[:],
                                                 tiles[kt][:, qs * P:(qs + 1) * P],
                                                 vres[:, kt, h * VW:(h + 1) * VW],
                                                 start=(kt == 0), stop=(kt == qa))
                            rcp = zpool.tile([P, 1], F32, tag='rcp',
                                             name=f'rcp_{qr}_{h}_{qs}')
                            nc.vector.reciprocal(rcp[:], zps[:, P:P + 1])
                            zsb = zpool.tile([P, P], BF16, tag='zsb',
                                             name=f'zsb_{qr}_{h}_{qs}')
                            nc.vector.tensor_scalar_mul(zsb[:], zps[:, 0:P], rcp[:])
                            # delay the transpose one step so the DVE epilogue
                            # hides under the next PV block's matmuls
                            pend_t.append((h, qa, zsb))
                            if len(pend_t) > 1:
                                flush_t()

                    def emit_s3(qr, ci):
                        ec, tl = divmod(ci, 4)
                        tt = 4 * qr + tl
                        es = slice(ec * 512, ec * 512 + 512)
                        pso = opp.tile([P, 512], F32, tag='pso', name=f'pso_{tt}_{ec}')
                        for h in range(HG):
                            nc.tensor.matmul(pso[:], zres[h][:, tt * P:(tt + 1) * P],
                                             wo_sb[:, ec, h, :],
                                             start=(h == 0), stop=(h == HG - 1))
                        osb = ospool.tile([P, 512], BF16, tag='osb',
                                          name=f'osb_{tt}_{ec}')
                        if (tt + ec) % 2 == 0:
                            nc.scalar.copy(osb[:], pso[:])
                        else:
                            nc.vector.tensor_copy(osb[:], pso[:])
                        nc.sync.dma_start(partial[tt][:, es], osb[:])

                    def emit_rs(qr):
                        # Megatron all-reduce fused with token-halving, emitted
                        # per query-chunk so it overlaps later chunks' compute.
                        # Pair rank 0 keeps tiles (4qr, 4qr+1), rank 1 keeps
                        # (4qr+2, 4qr+3); the host stitches accordingly.
                        nc.gpsimd.collective_compute(
                            'ReduceScatter', ADD, replica_groups=PAIRS,
                            ins=[partial[4 * qr:4 * qr + 4].opt()],
                            outs=[rso[2 * qr:2 * qr + 2].opt()])
                        nc.sync.dma_start(out[2 * qr:2 * qr + 2],
                                          rso[2 * qr:2 * qr + 2])

                    jobs = [(qr, h) for qr in range(NC512) for h in range(HG)]
                    emit_qk(*jobs[0])
                    for i, (qr, h) in enumerate(jobs):
                        if i + 1 < len(jobs):
                            emit_qk(*jobs[i + 1])
                        emit_pv(qr, h)
                        if qr >= 1:
                            emit_s3(qr - 1, 2 * h)
                            emit_s3(qr - 1, 2 * h + 1)
                            if h == HG - 1:
                                emit_rs(qr - 1)
                    while pend_t:
                        flush_t()
                    for ci in range(16):
                        emit_s3(NC512 - 1, ci)
                    emit_rs(NC512 - 1)

    nc.compile()
    return nc


def _host_inputs(x, Wqkv, Wo):
    """Build the 8 per-core input maps (each core gets a distinct shard)."""
    BF = ml_dtypes.bfloat16
    # RoPE tables (f32 math, bf16 storage; sign folded into sinP rows 0:64)
    inv_freq = (1.0 / (BASE ** (np.arange(0, DH, 2, dtype=np.float32) / DH))).astype(np.float32)
    t = np.arange(S, dtype=np.float32)
    freqs = np.einsum('i,j->ij', t, inv_freq).astype(np.float32)   # [S, 64]
    emb = np.concatenate([freqs, freqs], axis=-1)                   # [S, 128]
    cosT = np.ascontiguousarray(np.cos(emb).T).astype(BF)           # [128, S]
    sinT = np.ascontiguousarray(np.sin(emb).T)
    sinT[0:64] = -sinT[0:64]
    sinP = sinT.astype(BF)

    # triangular causal mask [128, 128]: keep iff k_rel <= q_rel
    maskT = (np.arange(P)[:, None] <= np.arange(P)[None, :]).astype(BF)

    def xT_batch(b):
        # chunk-major [tcn, 128, 16, 512] so each chunk is one contiguous DMA
        # (cast to bf16 first so the transpose copy moves half the bytes)
        return np.ascontiguousarray(
            x[b].astype(BF).T.reshape(NDM, P, NC512, 512).transpose(2, 1, 0, 3))

    def wqkT_group(g):
        heads = range(HG * g, HG * g + HG)
        blocks = [Wqkv[h * DH:(h + 1) * DH] for h in heads] + \
                 [Wqkv[D + h * DH:D + (h + 1) * DH] for h in heads]
        return np.stack([
            np.ascontiguousarray(
                blk.astype(BF).T.reshape(NDM, P, P).transpose(1, 0, 2))
            for blk in blocks
        ])                                                       # [16, 128, 16, 128]

    def wvT_group(g):
        heads = range(HG * g, HG * g + HG)
        Wv = np.concatenate([Wqkv[2 * D + h * DH:2 * D + (h + 1) * DH] for h in heads])
        return np.ascontiguousarray(
            Wv.astype(BF).T.reshape(NDM, P, 4, 256).transpose(2, 1, 0, 3))

    def woT_group(g):
        Wog = Wo[:, g * HG * DH:(g + 1) * HG * DH]               # [D, 1024]
        return np.ascontiguousarray(
            Wog.astype(BF).T.reshape(HG, P, NC512, 512).transpose(2, 1, 0, 3))

    with ThreadPoolExecutor(max_workers=8) as ex:
        xTs = list(ex.map(xT_batch, range(B)))
        wqkTs = list(ex.map(wqkT_group, range(2)))
        wvTs = list(ex.map(wvT_group, range(2)))
        woTs = list(ex.map(woT_group, range(2)))

    in_maps = []
    for c in range(8):
        b, g = c // 2, c % 2
        p = b                        # rank slot of this core within its group AG
        cs = np.stack([cosT[:, c * 256:(c + 1) * 256],
                       sinP[:, c * 256:(c + 1) * 256]])
        in_maps.append({
            'xT_in': xTs[b][2 * g:2 * g + 2],
            'wqk_in': wqkTs[g][4 * p:4 * p + 4],
            'wv_in': wvTs[g][p:p + 1],
            'wo_in': woTs[g][p:p + 1],
            'cs_in': cs,
            'maskT': maskT,
        })
    return in_maps


def kernel(x, Wqkv, Wo):
    from concourse.bass_utils import run_bass_kernel_spmd

    if 'nc' not in _CACHE:
        _CACHE['nc'] = _build_program()
    nc = _CACHE['nc']

    in_maps = _host_inputs(np.asarray(x, dtype=np.float32),
                           np.asarray(Wqkv, dtype=np.float32),
                           np.asarray(Wo, dtype=np.float32))
    res = run_bass_kernel_spmd(nc, in_maps, core_ids=list(range(8)))
    full = np.empty((B, S, D), dtype=np.float32)
    for b in range(B):
        fb = full[b].reshape(NC512, 4, P, D)
        fb[:, 0:2] = res.results[2 * b]['out'].reshape(NC512, 2, P, D)
        fb[:, 2:4] = res.results[2 * b + 1]['out'].reshape(NC512, 2, P, D)
    return full
